# revision 11
# baseline (speedup 1.0000x reference)
"""Two-layer GCN (ActorGCN) on 8 Trainium2 NeuronCores.

Strategy (graph-partition sharding, per the problem's sharding hint):
  * Nodes are assigned to 8 cores (balanced by in-degree), and within each
    core packed into NW windows of <=128 nodes balanced by in-edge count.
  * Each core processes exactly the edges whose *destination* lies in its
    windows.  The message gather xw[src] is served by a core-private full
    copy of the (pre-scaled) feature table in DRAM via the custom
    InstDMAGatherAnt SWDGE gather (int16 indices -> the table is addressed
    in 1KB "quad rows" of 4 nodes; an edge's source node selects one of 4
    column classes so indices stay < 32768).
  * The scatter-add (segment sum by dst) is a one-hot matmul: for each
    128-edge chunk, DVE builds onehot[e, d] = (dstl[e]==d) * dis[dst[e]]
    and the PE accumulates psum[feat, dst] += msg[e,:]^T onehot over the
    window's 16 chunks.  Symmetric-norm scaling is folded in: the table
    rows are pre-scaled by dis[src], the onehot carries dis[dst].
  * Layer boundary: cores exchange their slice of the layer-2 table via a
    host gather between two NEFF launches (all structure is static, so the
    two programs are built and compiled once and cached).

Everything data-dependent (which edge goes to which slot) lives in the
*inputs* (index/metadata arrays built on the host); the Bass programs are
fully static and shared by all 8 cores.
"""

import sys
import numpy as np

sys.path.insert(0, "/opt/trn_rl_repo")

# ----------------------------------------------------------------------------
# configuration
# ----------------------------------------------------------------------------

N = 100_000
E = 1_600_000
F_IN = 14
F1 = F_IN + 1            # +stem feature
H = 64
NB = 105                 # block logits
NS = 105                 # stem logits
NO = NB + NS             # 210
KEEP = 0.8

NC = 8                   # cores
SH = N // NC             # 12500 nodes/core
NW = 108                 # windows per core
WPOS = 128               # positions (dst slots) per window
CELL = 512               # edge slots per (window, class) cell
WSLOT = 4 * CELL         # 2048 edge slots per window
NPAIR = NW // 2          # window pairs (one gather call per (pair, class))
ES_C = NW * WSLOT        # edge slots per core (221184)
NCH = ES_C // 128        # onehot chunks per core (1728)
SLOTS_C = NW * WPOS      # node slots per core (13824)
ST = NC * SLOTS_C        # total node slots (110592)
TR = ST // 4             # gather-table quad rows (27648) -- int16 safe

_PROGS = {}


def _reconfig(**kw):
    """Test hook: shrink the problem (recompute derived sizes)."""
    g = globals()
    g.update(kw)
    g["F1"] = g["F_IN"] + 1
    g["NO"] = g["NB"] + g["NS"]
    g["SH"] = g["N"] // g["NC"]
    g["NPAIR"] = g["NW"] // 2
    g["WSLOT"] = 4 * g["CELL"]
    g["ES_C"] = g["NW"] * g["WSLOT"]
    g["NCH"] = g["ES_C"] // 128
    g["SLOTS_C"] = g["NW"] * g["WPOS"]
    g["ST"] = g["NC"] * g["SLOTS_C"]
    g["TR"] = g["ST"] // 4
    _PROGS.clear()


# ----------------------------------------------------------------------------
# program builders
# ----------------------------------------------------------------------------

def _quad_view(ap, g):
    """View a [ST, H] f32 DRAM AP as [TR, 256] rows and take class-g columns."""
    quad = ap.rearrange("(r k) f -> r (k f)", k=4)
    return quad[:, 64 * g:64 * (g + 1)]


def _emit_scatter_layer(nc, tc, pools, xws_ap, idx_sb, dstl_sb, disd_sb,
                        iota_sb, bcol_sb, dropw_dram, epilogue):
    """Emit the gather + one-hot scatter for one GCN layer.

    For each window pair p and class g a single 1024-index dma_gather call
    fetches the edge messages; 8 onehot matmuls (4 per window) accumulate
    them into the pair's two psum tiles.  `epilogue(w, h_tile)` consumes the
    finished [64, 128] relu(S+b)*drop tile of window w.
    """
    import concourse.mybir as mybir

    gpool, ohpool, pspool, hpool, dwpool = pools
    for p in range(NPAIR):
        ps = [pspool.tile([64, WPOS], mybir.dt.float32, tag=f"ps{i}", name=f"ps{i}_{p}")
              for i in range(2)]
        acc = [0, 0]
        for g in range(4):
            gt = gpool.tile([128, 8, H], mybir.dt.float32, tag="g")
            base16 = (p * 4096 + g * 1024) // 16
            nc.gpsimd.dma_gather(
                out_ap=gt[:],
                in_ap=_quad_view(xws_ap, g),
                idxs_ap=idx_sb[:, base16:base16 + 64],
                num_idxs=1024,
                num_idxs_reg=1024,
                elem_size=H,
                elem_step=4 * H,
            )
            for k in range(8):
                ch = p * 32 + g * 8 + k
                wi = k // 4
                oh = ohpool.tile([128, WPOS], mybir.dt.float32, tag="oh")
                nc.vector.tensor_scalar(
                    out=oh[:], in0=iota_sb[:],
                    scalar1=dstl_sb[:, ch:ch + 1],
                    scalar2=disd_sb[:, ch:ch + 1],
                    op0=mybir.AluOpType.is_equal,
                    op1=mybir.AluOpType.mult,
                )
                a = acc[wi]
                nc.tensor.matmul(
                    out=ps[wi][:], lhsT=gt[:, k, :], rhs=oh[:],
                    start=(a == 0), stop=(a == 15),
                )
                acc[wi] += 1
        for wi in range(2):
            w = 2 * p + wi
            h_t = hpool.tile([65, WPOS], mybir.dt.float32, tag="h")
            nc.scalar.activation(h_t[0:64, :], ps[wi][:],
                                 mybir.ActivationFunctionType.Relu,
                                 bias=bcol_sb[:])
            dw = dwpool.tile([64, WPOS], mybir.dt.float32, tag="dw")
            nc.sync.dma_start(out=dw[:], in_=dropw_dram[w])
            nc.vector.tensor_tensor(out=h_t[0:64, :], in0=h_t[0:64, :],
                                    in1=dw[:], op=mybir.AluOpType.mult)
            epilogue(w, h_t)


def _build_launch_a():
    import concourse.bacc as bacc
    import concourse.mybir as mybir
    from concourse.tile import TileContext

    f32 = mybir.dt.float32
    nc = bacc.Bacc("TRN2", target_bir_lowering=False, debug=False,
                   num_devices=NC)
    h1t = nc.dram_tensor("h1t", [F1, ST], f32, kind="ExternalInput").ap()
    w1 = nc.dram_tensor("w1", [F1, H], f32, kind="ExternalInput").ap()
    w2 = nc.dram_tensor("w2", [H, H], f32, kind="ExternalInput").ap()
    idx16 = nc.dram_tensor("idx16", [128, ES_C // 16], mybir.dt.int16,
                           kind="ExternalInput").ap()
    dstl = nc.dram_tensor("dstl", [128, NCH], f32, kind="ExternalInput").ap()
    disd = nc.dram_tensor("disd", [128, NCH], f32, kind="ExternalInput").ap()
    disw = nc.dram_tensor("disw", [128, NW], f32, kind="ExternalInput").ap()
    dropw = nc.dram_tensor("dropw", [NW, H, WPOS], f32, kind="ExternalInput").ap()
    bcol = nc.dram_tensor("bcol", [H, 1], f32, kind="ExternalInput").ap()
    iota = nc.dram_tensor("iota", [128, WPOS], f32, kind="ExternalInput").ap()
    xws2s = nc.dram_tensor("xws2s", [SLOTS_C, H], f32, kind="ExternalOutput").ap()
    xws1 = nc.dram_tensor("xws1", [ST, H], f32).ap()

    with TileContext(nc) as tc:
        with tc.tile_pool(name="const", bufs=1) as cpool, \
             tc.tile_pool(name="slab", bufs=2) as slabpool, \
             tc.tile_pool(name="x1", bufs=3) as x1pool, \
             tc.tile_pool(name="psx", bufs=2, space="PSUM") as psxpool, \
             tc.tile_pool(name="g", bufs=4) as gpool, \
             tc.tile_pool(name="oh", bufs=6) as ohpool, \
             tc.tile_pool(name="ps", bufs=2, space="PSUM") as pspool, \
             tc.tile_pool(name="ps2", bufs=2, space="PSUM") as ps2pool, \
             tc.tile_pool(name="h", bufs=3) as hpool, \
             tc.tile_pool(name="dw", bufs=3) as dwpool, \
             tc.tile_pool(name="x2", bufs=3) as x2pool:

            w1_sb = cpool.tile([F1, H], f32)
            nc.sync.dma_start(out=w1_sb[:], in_=w1[:])
            w2_sb = cpool.tile([H, H], f32)
            nc.sync.dma_start(out=w2_sb[:], in_=w2[:])
            iota_sb = cpool.tile([128, WPOS], f32)
            nc.sync.dma_start(out=iota_sb[:], in_=iota[:])
            bcol_sb = cpool.tile([H, 1], f32)
            nc.sync.dma_start(out=bcol_sb[:], in_=bcol[:])
            disw_sb = cpool.tile([128, NW], f32)
            nc.sync.dma_start(out=disw_sb[:], in_=disw[:])
            idx_sb = cpool.tile([128, ES_C // 16], mybir.dt.int16)
            nc.sync.dma_start(out=idx_sb[:], in_=idx16[:])
            dstl_sb = cpool.tile([128, NCH], f32)
            nc.sync.dma_start(out=dstl_sb[:], in_=dstl[:])
            disd_sb = cpool.tile([128, NCH], f32)
            nc.sync.dma_start(out=disd_sb[:], in_=disd[:])

            # phase 1: xws1[slot] = (h1s[slot] @ W1)  (dis pre-folded on host)
            CHUNKS = ST // 128
            SLAB_CH = 54                 # chunks per staged slab
            for s0 in range(0, CHUNKS, SLAB_CH):
                nch = min(SLAB_CH, CHUNKS - s0)
                slab = slabpool.tile([F1, SLAB_CH * 128], f32, tag="slab")
                nc.sync.dma_start(out=slab[:, :nch * 128],
                                  in_=h1t[:, s0 * 128:(s0 + nch) * 128])
                for k in range(nch):
                    psx = psxpool.tile([128, H], f32, tag="psx")
                    nc.tensor.matmul(out=psx[:], lhsT=slab[:, k * 128:(k + 1) * 128],
                                     rhs=w1_sb[:], start=True, stop=True)
                    x1 = x1pool.tile([128, H], f32, tag="x1")
                    nc.scalar.activation(x1[:], psx[:],
                                         mybir.ActivationFunctionType.Copy)
                    row = (s0 + k) * 128
                    nc.sync.dma_start(out=xws1[row:row + 128, :], in_=x1[:])

            # phase 2: layer-1 conv; epilogue computes xws2 shard rows
            def epilogue(w, h_t):
                ps2 = ps2pool.tile([WPOS, H], f32, tag="ps2")
                nc.tensor.matmul(out=ps2[:], lhsT=h_t[0:64, :], rhs=w2_sb[:],
                                 start=True, stop=True)
                x2 = x2pool.tile([WPOS, H], f32, tag="x2")
                nc.vector.tensor_scalar(
                    out=x2[:], in0=ps2[:],
                    scalar1=disw_sb[:, w:w + 1], scalar2=None,
                    op0=mybir.AluOpType.mult)
                nc.sync.dma_start(out=xws2s[w * WPOS:(w + 1) * WPOS, :], in_=x2[:])

            _emit_scatter_layer(nc, tc, (gpool, ohpool, pspool, hpool, dwpool),
                                xws1, idx_sb, dstl_sb, disd_sb, iota_sb,
                                bcol_sb, dropw, epilogue)
    nc.compile()
    return nc


def _build_launch_b():
    import concourse.bacc as bacc
    import concourse.mybir as mybir
    from concourse.tile import TileContext

    f32 = mybir.dt.float32
    nc = bacc.Bacc("TRN2", target_bir_lowering=False, debug=False,
                   num_devices=NC)
    xws2 = nc.dram_tensor("xws2", [ST, H], f32, kind="ExternalInput").ap()
    idx16 = nc.dram_tensor("idx16", [128, ES_C // 16], mybir.dt.int16,
                           kind="ExternalInput").ap()
    dstl = nc.dram_tensor("dstl", [128, NCH], f32, kind="ExternalInput").ap()
    disd = nc.dram_tensor("disd", [128, NCH], f32, kind="ExternalInput").ap()
    dropw = nc.dram_tensor("dropw", [NW, H, WPOS], f32, kind="ExternalInput").ap()
    bcol = nc.dram_tensor("bcol", [H, 1], f32, kind="ExternalInput").ap()
    iota = nc.dram_tensor("iota", [128, WPOS], f32, kind="ExternalInput").ap()
    wbs = nc.dram_tensor("wbs", [H + 1, NO], f32, kind="ExternalInput").ap()
    gbs = nc.dram_tensor("gbs", [NW, WPOS, NO], f32, kind="ExternalInput").ap()
    out_lg = nc.dram_tensor("out_lg", [NW, WPOS, NO], f32, kind="ExternalOutput").ap()
    out_sel = nc.dram_tensor("out_sel", [NW, WPOS, NO], f32, kind="ExternalOutput").ap()

    with TileContext(nc) as tc:
        with tc.tile_pool(name="const", bufs=1) as cpool, \
             tc.tile_pool(name="g", bufs=4) as gpool, \
             tc.tile_pool(name="oh", bufs=6) as ohpool, \
             tc.tile_pool(name="ps", bufs=2, space="PSUM") as pspool, \
             tc.tile_pool(name="psh", bufs=2, space="PSUM") as pshpool, \
             tc.tile_pool(name="h", bufs=3) as hpool, \
             tc.tile_pool(name="dw", bufs=3) as dwpool, \
             tc.tile_pool(name="gb", bufs=3) as gbpool, \
             tc.tile_pool(name="z", bufs=3) as zpool, \
             tc.tile_pool(name="lg", bufs=3) as lgpool, \
             tc.tile_pool(name="sel", bufs=3) as selpool, \
             tc.tile_pool(name="sm", bufs=4) as smpool:

            iota_sb = cpool.tile([128, WPOS], f32)
            nc.sync.dma_start(out=iota_sb[:], in_=iota[:])
            bcol_sb = cpool.tile([H, 1], f32)
            nc.sync.dma_start(out=bcol_sb[:], in_=bcol[:])
            wbs_sb = cpool.tile([H + 1, NO], f32)
            nc.sync.dma_start(out=wbs_sb[:], in_=wbs[:])
            idx_sb = cpool.tile([128, ES_C // 16], mybir.dt.int16)
            nc.sync.dma_start(out=idx_sb[:], in_=idx16[:])
            dstl_sb = cpool.tile([128, NCH], f32)
            nc.sync.dma_start(out=dstl_sb[:], in_=dstl[:])
            disd_sb = cpool.tile([128, NCH], f32)
            nc.sync.dma_start(out=disd_sb[:], in_=disd[:])

            def epilogue(w, h_t):
                nc.vector.memset(h_t[64:65, :], 1.0)
                psh = pshpool.tile([WPOS, NO], f32, tag="psh")
                nc.tensor.matmul(out=psh[:], lhsT=h_t[:], rhs=wbs_sb[:],
                                 start=True, stop=True)
                gb_t = gbpool.tile([WPOS, NO], f32, tag="gb")
                nc.sync.dma_start(out=gb_t[:], in_=gbs[w])
                lg_t = lgpool.tile([WPOS, NO], f32, tag="lg")
                nc.scalar.activation(lg_t[:], psh[:],
                                     mybir.ActivationFunctionType.Copy)
                nc.sync.dma_start(out=out_lg[w], in_=lg_t[:])
                z_t = zpool.tile([WPOS, NO], f32, tag="z")
                nc.vector.tensor_tensor(out=z_t[:], in0=psh[:], in1=gb_t[:],
                                        op=mybir.AluOpType.add)
                sel_t = selpool.tile([WPOS, NO], f32, tag="sel")
                for h0 in (0, NB):
                    zz = z_t[:, h0:h0 + NB]
                    nmx = smpool.tile([WPOS, 1], f32, tag="nmx")
                    nc.vector.tensor_reduce(nmx[:], zz, axis=mybir.AxisListType.X,
                                            op=mybir.AluOpType.max, negate=True)
                    ex = smpool.tile([WPOS, NB], f32, tag="ex")
                    nc.scalar.activation(ex[:], zz,
                                         mybir.ActivationFunctionType.Exp,
                                         bias=nmx[:])
                    sm = smpool.tile([WPOS, 1], f32, tag="sm")
                    nc.vector.tensor_reduce(sm[:], ex[:], axis=mybir.AxisListType.X,
                                            op=mybir.AluOpType.add)
                    rc = smpool.tile([WPOS, 1], f32, tag="rc")
                    nc.vector.reciprocal(rc[:], sm[:])
                    nc.vector.tensor_scalar(
                        out=sel_t[:, h0:h0 + NB], in0=ex[:],
                        scalar1=rc[:], scalar2=None,
                        op0=mybir.AluOpType.mult)
                nc.sync.dma_start(out=out_sel[w], in_=sel_t[:])

            _emit_scatter_layer(nc, tc, (gpool, ohpool, pspool, hpool, dwpool),
                                xws2, idx_sb, dstl_sb, disd_sb, iota_sb,
                                bcol_sb, dropw, epilogue)
    nc.compile()
    return nc


def _get_programs():
    if "a" not in _PROGS:
        _PROGS["a"] = _build_launch_a()
        _PROGS["b"] = _build_launch_b()
    return _PROGS["a"], _PROGS["b"]


# ----------------------------------------------------------------------------
# host-side graph preprocessing
# ----------------------------------------------------------------------------

def _snake_bins(n, nbins):
    """bins[i] for ranks 0..n-1: round-robin with direction reversal."""
    idx = np.arange(n)
    rows, cols = idx // nbins, idx % nbins
    return np.where(rows % 2 == 0, cols, nbins - 1 - cols).astype(np.int32)


def _prep_structure(src, dst):
    """All edge/permutation metadata.  src/dst include self-loops."""
    indeg = np.bincount(dst, minlength=N).astype(np.int64)  # includes loops
    dis = (1.0 / np.sqrt(indeg)).astype(np.float32)

    order = np.argsort(-indeg, kind="stable")
    core = np.empty(N, dtype=np.int32)
    core[order] = _snake_bins(N, NC)
    # windows within a core: snake over per-core degree order
    win = np.empty(N, dtype=np.int32)
    for c in range(NC):
        nodes_c = np.nonzero(core == c)[0]
        order_c = nodes_c[np.argsort(-indeg[nodes_c], kind="stable")]
        win[order_c] = _snake_bins(order_c.shape[0], NW)
    wing = core * NW + win                          # global window id

    # per-window in-edge load (for sanity)
    wload = np.bincount(wing[dst], minlength=NC * NW)
    assert wload.max() <= WSLOT - 64, f"window overload {wload.max()}"

    # ---- color balancing (src side) -------------------------------------
    # order edges by src for per-node target lookup
    es = np.argsort(src, kind="stable")
    src_s, dst_s = src[es], dst[es]
    starts = np.searchsorted(src_s, np.arange(N + 1))
    tgt_win = wing[dst_s]

    cell = np.zeros((NC * NW, 4), dtype=np.int32)
    quota = np.zeros((NC * NW, 4), dtype=np.int32)
    # color quotas per window: WPOS/4 positions per color class
    quota[:] = WPOS // 4
    color = np.empty(N, dtype=np.int8)
    rng = np.random.default_rng(1234)
    visit = rng.permutation(N)
    for v in visit:
        t = tgt_win[starts[v]:starts[v + 1]]
        w = wing[v]
        if t.shape[0]:
            tw, tc = np.unique(t, return_counts=True)
            load = cell[tw]                           # [k, 4]
            over = (load + tc[:, None] > CELL).any(axis=0)
            score = load.sum(axis=0) + over * 10**9
        else:
            score = np.zeros(4)
        score = score + np.where(quota[w] > 0, 0, 10**12)
        g = int(np.argmin(score))
        color[v] = g
        quota[w, g] -= 1
        if t.shape[0]:
            np.add.at(cell, (t, int(g)), 1)
    assert cell.max() <= CELL, f"cell overflow {cell.max()}"

    # positions within windows: color g gets positions with pos%4==g
    pos = np.empty(N, dtype=np.int32)
    wkey = wing.astype(np.int64) * 4 + color
    ordern = np.argsort(wkey, kind="stable")
    # rank within (window, color)
    key_sorted = wkey[ordern]
    firsts = np.r_[0, np.nonzero(np.diff(key_sorted))[0] + 1]
    rank = np.arange(N) - np.repeat(firsts, np.diff(np.r_[firsts, N]))
    pos[ordern] = rank * 4 + (key_sorted % 4).astype(np.int32)
    assert pos.max() < WPOS
    rho = core.astype(np.int64) * SLOTS_C + win.astype(np.int64) * WPOS + pos

    # ---- edge slot assignment -------------------------------------------
    cw = wing[dst].astype(np.int64)                  # target window per edge
    cg = color[src].astype(np.int64)                 # class per edge
    cellid = cw * 4 + cg
    ek = np.argsort(cellid * (4 * TR) + (rho[src] >> 2), kind="stable")
    cid_s = cellid[ek]
    cfirst = np.searchsorted(cid_s, np.arange(NC * NW * 4 + 1))
    crank = np.arange(cid_s.shape[0]) - np.repeat(
        cfirst[:-1], np.diff(cfirst))
    # slot base of each cell inside its core's edge-slot array
    wl = cid_s // 4 % NW                             # window within core
    gl = cid_s % 4
    pairl = wl // 2
    slot_in_core = pairl * 4096 + gl * 1024 + (wl % 2) * CELL + crank
    ecore = cid_s // (4 * NW)

    idx_lin = np.zeros((NC, ES_C), dtype=np.int16)
    dstl_lin = np.full((NC, ES_C), -1.0, dtype=np.float32)
    disd_lin = np.zeros((NC, ES_C), dtype=np.float32)
    su, du = src[ek], dst[ek]
    idx_lin[ecore, slot_in_core] = (rho[su] >> 2).astype(np.int16)
    dstl_lin[ecore, slot_in_core] = pos[du].astype(np.float32)
    disd_lin[ecore, slot_in_core] = dis[du]

    idx16 = np.ascontiguousarray(
        np.tile(idx_lin.reshape(NC, ES_C // 16, 16).transpose(0, 2, 1),
                (1, 8, 1)))
    dstl = np.ascontiguousarray(dstl_lin.reshape(NC, NCH, 128).transpose(0, 2, 1))
    disd = np.ascontiguousarray(disd_lin.reshape(NC, NCH, 128).transpose(0, 2, 1))
    return dis, core, win, pos, rho, idx16, dstl, disd


def _window_scatter(values, core, win, pos, shape_tail):
    """Scatter per-node rows into [NC, NW, WPOS, *tail] (zeros elsewhere)."""
    out = np.zeros((NC, NW, WPOS) + shape_tail, dtype=np.float32)
    out[core, win, pos] = values
    return out


# ----------------------------------------------------------------------------
# persistent PJRT runner (jit once, repeatable timed runs)
# ----------------------------------------------------------------------------

class _Runner:
    def __init__(self, nc, n_cores):
        import jax
        import concourse.mybir as mybir
        from jax.sharding import Mesh, PartitionSpec
        from jax.experimental.shard_map import shard_map
        from concourse.bass2jax import (_bass_exec_p, install_neuronx_cc_hook,
                                        partition_id_tensor)
        install_neuronx_cc_hook()
        self.jax = jax
        self.n_cores = n_cores
        pname = nc.partition_id_tensor.name if nc.partition_id_tensor else None
        in_names, out_names, out_avals, zero_outs = [], [], [], []
        for alloc in nc.m.functions[0].allocations:
            if not isinstance(alloc, mybir.MemoryLocationSet):
                continue
            name = alloc.memorylocations[0].name
            if alloc.kind == "ExternalInput":
                if name != pname:
                    in_names.append(name)
            elif alloc.kind == "ExternalOutput":
                shape = tuple(alloc.tensor_shape)
                dtype = mybir.dt.np(alloc.dtype)
                out_names.append(name)
                out_avals.append(jax.core.ShapedArray(shape, dtype))
                zero_outs.append(np.zeros(shape, dtype))
        self.in_names, self.out_names = in_names, out_names
        self.out_avals, self.zero_outs = out_avals, zero_outs
        all_in = list(in_names) + list(out_names)
        if pname is not None:
            all_in.append(pname)

        def _body(*args):
            operands = list(args)
            if pname is not None:
                operands.append(partition_id_tensor())
            return tuple(_bass_exec_p.bind(
                *operands, out_avals=tuple(out_avals), in_names=tuple(all_in),
                out_names=tuple(out_names), lowering_input_output_aliases=(),
                sim_require_finite=True, sim_require_nnan=True, nc=nc))

        devices = jax.devices()[:n_cores]
        mesh = Mesh(np.asarray(devices), ("core",))
        n_io = len(in_names) + len(out_names)
        self.fn = jax.jit(
            shard_map(_body, mesh=mesh,
                      in_specs=(PartitionSpec("core"),) * n_io,
                      out_specs=(PartitionSpec("core"),) * len(out_names),
                      check_rep=False),
            keep_unused=True)
        self._zeros_dev = None

    def run(self, in_maps):
        jax = self.jax
        concat = [np.concatenate([np.asarray(in_maps[c][n])
                                  for c in range(self.n_cores)], axis=0)
                  for n in self.in_names]
        if self._zeros_dev is None:
            self._zeros_dev = [
                jax.device_put(np.zeros((self.n_cores * z.shape[0],
                                         *z.shape[1:]), z.dtype))
                for z in self.zero_outs]
        staged = [jax.device_put(a) for a in concat] + self._zeros_dev
        import time as _t
        t0 = _t.perf_counter()
        outs = self.fn(*staged)
        jax.block_until_ready(outs)
        dt = _t.perf_counter() - t0
        res = []
        for c in range(self.n_cores):
            d = {}
            for i, name in enumerate(self.out_names):
                d[name] = np.asarray(outs[i]).reshape(
                    self.n_cores, *self.out_avals[i].shape)[c]
            res.append(d)
        return res, dt


_RUNNERS = {}
_LAST_EXEC_NS = None


def _get_runners():
    nc_a, nc_b = _get_programs()
    if "a" not in _RUNNERS:
        _RUNNERS["a"] = _Runner(nc_a, NC)
        _RUNNERS["b"] = _Runner(nc_b, NC)
    return _RUNNERS["a"], _RUNNERS["b"]


# ----------------------------------------------------------------------------
# main entry
# ----------------------------------------------------------------------------

def kernel(x, W1, b1, W2, b2, Wb, bb, Ws, bs, drop1, drop2, gb, gs,
           edge_index, stem_idxs):

    x = np.asarray(x, dtype=np.float32)
    src0 = np.asarray(edge_index[0], dtype=np.int64)
    dst0 = np.asarray(edge_index[1], dtype=np.int64)
    loops = np.arange(N, dtype=np.int64)
    src = np.concatenate([src0, loops])
    dst = np.concatenate([dst0, loops])

    dis, core, win, pos, rho, idx16, dstl, disd = _prep_structure(src, dst)

    stem = np.zeros((N,), dtype=np.float32)
    stem[np.asarray(stem_idxs, dtype=np.int64)] = 1.0
    h1 = np.concatenate([x, stem[:, None]], axis=1)          # [N, 15]
    h1s = h1 * dis[:, None]
    h1t = np.zeros((F1, ST), dtype=np.float32)
    h1t[:, rho] = h1s.T

    disw = np.zeros((NC, 128, NW), dtype=np.float32)
    disw[core, pos, win] = dis
    dropw1 = _window_scatter(np.asarray(drop1, np.float32), core, win, pos, (H,))
    dropw1 = np.ascontiguousarray(dropw1.transpose(0, 1, 3, 2))   # [NC,NW,H,WPOS]
    dropw2 = _window_scatter(np.asarray(drop2, np.float32), core, win, pos, (H,))
    dropw2 = np.ascontiguousarray(dropw2.transpose(0, 1, 3, 2))
    gbs_full = _window_scatter(
        np.concatenate([np.asarray(gb, np.float32), np.asarray(gs, np.float32)],
                       axis=1), core, win, pos, (NO,))

    wbs = np.concatenate(
        [np.concatenate([np.asarray(Wb, np.float32), np.asarray(Ws, np.float32)], axis=1),
         np.concatenate([np.asarray(bb, np.float32), np.asarray(bs, np.float32)])[None, :]],
        axis=0)                                               # [65, 210]
    iota = np.tile(np.arange(WPOS, dtype=np.float32), (128, 1))
    b1col = np.asarray(b1, np.float32).reshape(H, 1)
    b2col = np.asarray(b2, np.float32).reshape(H, 1)

    run_a, run_b = _get_runners()

    maps_a = [{
        "h1t": h1t, "w1": np.asarray(W1, np.float32),
        "w2": np.asarray(W2, np.float32),
        "idx16": idx16[c], "dstl": dstl[c], "disd": disd[c],
        "disw": disw[c], "dropw": dropw1[c], "bcol": b1col, "iota": iota,
    } for c in range(NC)]
    res_a, dt_a = run_a.run(maps_a)
    xws2 = np.concatenate([res_a[c]["xws2s"] for c in range(NC)], axis=0)

    maps_b = [{
        "xws2": xws2, "idx16": idx16[c], "dstl": dstl[c], "disd": disd[c],
        "dropw": dropw2[c], "bcol": b2col, "iota": iota, "wbs": wbs,
        "gbs": gbs_full[c],
    } for c in range(NC)]
    res_b, dt_b = run_b.run(maps_b)
    globals()["_LAST_EXEC_NS"] = int((dt_a + dt_b) * 1e9)

    lg = np.stack([res_b[c]["out_lg"] for c in range(NC)])   # [NC,NW,WPOS,NO]
    sel = np.stack([res_b[c]["out_sel"] for c in range(NC)])
    block_logits = lg[core, win, pos, :NB]
    stem_logits = lg[core, win, pos, NB:]
    selected_block = sel[core, win, pos, :NB]
    selected_stem = sel[core, win, pos, NB:]
    return (block_logits, stem_logits, selected_block, selected_stem)
